# revision 7
# baseline (speedup 1.0000x reference)
"""Trainium2 Bass kernel for nn_CausePredictor (RGCN + pairwise MLP).

Sharding: data-parallel over the pairwise row index i (dim 1 of the
[B,S,S] output): 8 cores x 25 rows, replicated over B=4.  All per-core
differences are encoded as input DATA, so one SPMD program serves all
cores.

Math (matching reference.py):
  h   = sum_k Ahat_k.T @ (x[b] @ basis_k) + x[b] @ root + bias
  u   = h @ (W1a*S1)   v = h @ (W1c*S1)          # S1 = fp8 range scale
  T   = pe_k @ (W1b*S1) + pe_v @ (W1d*S1)        # [11, 512], host-built
  h1  = u[j] + v[i] + T[pos(i,j)]                # scaled by S1
  rh1 = relu(h1) -> fp8e4m3
  g2  = rh1 @ (W2*S2) via fp8 DoubleRow + hybrid fp8 residual weights
  rh2 = relu(g2)/(S1*S2) -> bf16
  out = sigmoid(rh2 @ Wp) * mask                 # mask applied on host

Stage B engine split: DVE does the u+peR adds (tensor_tensor, 800-wide
2x mode) and most relu(+v)->fp8 stores (tensor_scalar 2x), Pool (gpsimd)
takes the rest, Act drains GEMM2 PSUM (relu+rescale, 800-wide APs
spanning two banks of a [128,2,512] psum tile) and runs the per-batch
sigmoid.  GEMM3 accumulates all 52 (b,unit) rows into one [52,400]
PSUM bank via sliding-window column-placed Wp weights.
"""

import sys

sys.path.insert(0, "/opt/trn_rl_repo")

import numpy as np

B, S, D, M, P = 4, 200, 300, 512, 100
NREL, MAXL = 9, 10
NCORES = 8
IPC = S // NCORES  # 25 rows of i per core
NU = IPC // 2 + 1  # 13 units per b: 12x 2-row + 1x 1-row
FPC = IPC * S  # 5000 pairs per (b, core)
FPAD = FPC + 200  # peR padded so unit 12 reads in-bounds
SC = S + IPC  # 225

S1 = 16.0  # scale baked into W1 (u/v/T) so rh1 fills fp8 range
S2 = 32.0  # scale baked into W2

# Number of mc PAIRS (2x128 contraction each) run in bf16 instead of
# fp8-DoubleRow: 0 = all fp8 (fastest), 2 = all bf16 (most accurate).
FSPLIT = 1

_prog_cache = {}

DCW = [128, 128, 44]  # D=300 chunks
JCW = [128, 72]  # S=200 chunks


def _rel_adj(s):
    ra = np.arange(s)[None, :] - np.arange(s)[:, None]
    for i in range(s):
        ra[i, i + 1 :] = 1
        num = 1
        for o in range(i - 1, -1, -2):
            ra[i, o] = -num
            if o - 1 >= 0:
                ra[i, o - 1] = -num
            num += 1
        ra[i, :i] = np.maximum(ra[i, :i], -8)
    return ra


def _pack_k(w):
    """[K, N] -> [128, ceil(K/128)*N], K chunked onto partitions."""
    k, n = w.shape
    nch = (k + 127) // 128
    out = np.zeros((128, nch * n), np.float32)
    for c in range(nch):
        r = min(128, k - c * 128)
        out[:r, c * n : c * n + n] = w[c * 128 : c * 128 + r]
    return out


def _build_program():
    import ml_dtypes  # noqa: F401
    import concourse.tile as tile
    from concourse import bacc, mybir

    f32 = mybir.dt.float32
    bf16 = mybir.dt.bfloat16
    f8 = mybir.dt.float8e4
    AF = mybir.ActivationFunctionType
    OP = mybir.AluOpType
    DR = mybir.MatmulPerfMode.DoubleRow

    nc = bacc.Bacc()

    dxT = nc.declare_dram_parameter("xT", [D, B * SC], bf16, isOutput=False)
    dahat = nc.declare_dram_parameter("ahat", [128, 4 * SC], bf16, isOutput=False)
    dbasis = nc.declare_dram_parameter("basis", [128, 1800], bf16, isOutput=False)
    droot = nc.declare_dram_parameter("root", [128, 900], bf16, isOutput=False)
    dbias = nc.declare_dram_parameter("bias", [128, 3], f32, isOutput=False)
    dw1a = nc.declare_dram_parameter("w1a", [128, 1536], bf16, isOutput=False)
    dw1c = nc.declare_dram_parameter("w1c", [128, 1536], bf16, isOutput=False)
    dw2q = nc.declare_dram_parameter("w2q", [128, 2 * 2 * M], f8, isOutput=False)
    dw2r = nc.declare_dram_parameter("w2r", [128, 2 * 2 * M], f8, isOutput=False)
    dw2b = nc.declare_dram_parameter("w2b", [128, 4 * M], bf16, isOutput=False)
    dwp = nc.declare_dram_parameter("wp", [128, 4 * 105], bf16, isOutput=False)
    dpeR = [nc.declare_dram_parameter(f"peR{g}", [128, 2 * FPAD], bf16,
                                      isOutput=False) for g in range(2)]
    dout = nc.declare_dram_parameter("out", [52, 400], f32, isOutput=True)

    with tile.TileContext(nc) as tc:
        with (
            tc.tile_pool(name="persist", bufs=1) as pp,
            tc.tile_pool(name="work", bufs=3) as pwork,
        ):
            def load(name, shape, dt, src):
                t = pp.tile(shape, dt, tag=name, name=name)
                nc.sync.dma_start(t[tuple(slice(None) for _ in shape)], src)
                return t

            xT = [load(f"xT{c}", [DCW[c], B * SC], bf16,
                       dxT[c * 128 : c * 128 + DCW[c], :]) for c in range(3)]
            basis = load("basis", [128, 1800], bf16, dbasis[:, :])
            root = load("root", [128, 900], bf16, droot[:, :])
            bias = load("bias", [128, 3], f32, dbias[:, :])
            w1a = load("w1a", [128, 1536], bf16, dw1a[:, :])
            w1c = load("w1c", [128, 1536], bf16, dw1c[:, :])
            ahat = load("ahat", [128, 4 * SC], bf16, dahat[:, :])
            wp = load("wp", [128, 4 * 105], bf16, dwp[:, :])
            w2q = pp.tile([128, 2, 2 * M], f8, tag="w2q", name="w2q")
            nc.sync.dma_start(w2q[:, :, :], dw2q[:, :])
            w2r = pp.tile([128, 2, 2 * M], f8, tag="w2r", name="w2r")
            nc.sync.dma_start(w2r[:, :, :], dw2r[:, :])
            w2b = load("w2b", [128, 4 * M], bf16, dw2b[:, :])
            peR = []
            for g in range(2):
                t = pp.tile([128, 2, FPAD], bf16, tag=f"peR{g}", name=f"peR{g}")
                nc.sync.dma_start(t[:, :, :], dpeR[g][:, :])
                peR.append(t)

            u2 = [[pp.tile([128, 2, S], bf16, tag=f"u2_{b}{g}", name=f"u2_{b}{g}")
                   for g in range(2)] for b in range(B)]
            vT = [[pp.tile([128, 26], f32, tag=f"vT{b}{mc}", name=f"vT{b}{mc}")
                   for mc in range(4)] for b in range(B)]
            hT = [[pp.tile([DCW[ec], SC], bf16, tag=f"hT{b}{ec}", name=f"hT{b}{ec}")
                   for ec in range(3)] for b in range(B)]
            t1 = [[[pp.tile([JCW[jc], D], bf16, tag=f"t1_{b}{k}{jc}",
                            name=f"t1_{b}{k}{jc}")
                    for jc in range(2)] for k in range(2)] for b in range(B)]
            sig = pp.tile([52, 400], f32, tag="sig", name="sig")

            # ---------------- stage A: t1 -> h -> u/v (all bf16) ----------
            with tc.tile_pool(name="psA", bufs=2, space="PSUM") as psA:
                for b in range(B):
                    for k in range(2):
                        for jc in range(2):
                            t1ps = psA.tile([JCW[jc], D], f32, tag="aps", name="t1ps")
                            for dc in range(3):
                                nc.tensor.matmul(
                                    t1ps[:, :],
                                    xT[dc][:, b * SC + jc * 128 : b * SC + jc * 128 + JCW[jc]],
                                    basis[0 : DCW[dc], (k * 3 + dc) * D : (k * 3 + dc) * D + D],
                                    start=(dc == 0), stop=(dc == 2),
                                )
                            if jc == 0:
                                nc.vector.tensor_copy(t1[b][k][jc][:, :], t1ps[:, :])
                            else:
                                nc.scalar.activation(t1[b][k][jc][:, :], t1ps[:, :],
                                                     AF.Copy)
                for b in range(B):
                    for ec in range(3):
                        hps = psA.tile([DCW[ec], SC], f32, tag="aps", name="hps")
                        first = True
                        for k in range(2):
                            for jc in range(2):
                                nc.tensor.matmul(
                                    hps[:, :],
                                    t1[b][k][jc][:, ec * 128 : ec * 128 + DCW[ec]],
                                    ahat[0 : JCW[jc], (k * 2 + jc) * SC : (k * 2 + jc + 1) * SC],
                                    start=first, stop=False)
                                first = False
                        for dc in range(3):
                            nc.tensor.matmul(
                                hps[:, :],
                                root[0 : DCW[dc], dc * D + ec * 128 : dc * D + ec * 128 + DCW[ec]],
                                xT[dc][:, b * SC : (b + 1) * SC],
                                start=False, stop=(dc == 2))
                        nc.scalar.activation(hT[b][ec][:, :], hps[:, :], AF.Identity,
                                             bias=bias[0 : DCW[ec], ec : ec + 1])
                for b in range(B):
                    for mc in range(4):
                        ups = psA.tile([128, SC], f32, tag="aps", name="ups")
                        for ec in range(3):
                            nc.tensor.matmul(
                                ups[:, 0:S],
                                w1a[0 : DCW[ec], ec * M + mc * 128 : ec * M + mc * 128 + 128],
                                hT[b][ec][:, 0:S], start=(ec == 0), stop=False)
                        for ec in range(3):
                            nc.tensor.matmul(
                                ups[:, S:SC],
                                w1c[0 : DCW[ec], ec * M + mc * 128 : ec * M + mc * 128 + 128],
                                hT[b][ec][:, S:SC], start=(ec == 0), stop=(ec == 2))
                        if mc % 2 == 0:
                            nc.vector.tensor_copy(u2[b][mc // 2][:, 0, :], ups[:, 0:S])
                        else:
                            nc.scalar.activation(u2[b][mc // 2][:, 1, :], ups[:, 0:S],
                                                 AF.Copy)
                        nc.vector.tensor_copy(vT[b][mc][:, 0:IPC], ups[:, S:SC])
                        nc.vector.memset(vT[b][mc][:, IPC:26], 0)

            # ---------------- stage B: the pairwise MLP ------------------
            with (
                tc.tile_pool(name="psB", bufs=3, space="PSUM") as psB,
                tc.tile_pool(name="psL", bufs=1, space="PSUM") as psL,
            ):
                pslogit = psL.tile([52, 512], f32, tag="pslogit", name="pslogit")
                nmm = 0
                NMM_TOT = B * NU * 4
                for b in range(B):
                    for u in range(NU):
                        # op1: rh1a[g] = u2-broadcast + peR slice  [128, 800]
                        rh1a = [pwork.tile([128, 800], bf16, tag=f"rh1a{g}",
                                           name=f"rh1a{g}") for g in range(2)]
                        for g in range(2):
                            nc.vector.tensor_tensor(
                                rh1a[g][:, :].rearrange("p (m r j) -> p m r j",
                                                        m=2, r=2),
                                u2[b][g][:, :, :].unsqueeze(2).broadcast_to(
                                    [128, 2, 2, S]),
                                peR[g][:, :, u * 400 : u * 400 + 400].rearrange(
                                    "p m (r j) -> p m r j", r=2),
                                OP.add)
                        # op2: rh1 = relu(rh1a + v), per (row, mc)
                        rh1f8 = [pwork.tile([128, 2, 400],
                                            f8 if g < 2 - FSPLIT else bf16,
                                            tag=f"rh1f8{g}", name=f"rh1f8{g}")
                                 for g in range(2)]
                        slot = 0
                        for g in range(2):
                            for m in range(2):
                                for h in range(2):
                                    mc = g * 2 + m
                                    eng = nc.vector if slot % 8 < 5 else nc.gpsimd
                                    eng.tensor_scalar(
                                        out=rh1f8[g][:, m, h * 200 : h * 200 + 200],
                                        in0=rh1a[g][:, m * 400 + h * 200 : m * 400 + h * 200 + 200],
                                        scalar1=vT[b][mc][:, 2 * u + h : 2 * u + h + 1],
                                        scalar2=0.0,
                                        op0=OP.add, op1=OP.max)
                                    slot += 1
                        # GEMM2 into [128, 2, 512] psum tiles (2 banks each)
                        big = [psB.tile([128, 2, 512], f32, tag="big", name="big")
                               for _ in range(2)]
                        for t in range(2):
                            for half in range(2):
                                n = t * 2 + half
                                dst = big[t][:, half, 0:400]
                                mms = []
                                for g in range(2):
                                    if g < 2 - FSPLIT:
                                        for w2f in (w2q, w2r):
                                            mms.append((w2f, g, True))
                                    else:
                                        for m in range(2):
                                            mms.append((None, g * 2 + m, False))
                                for i, (w2f, gm, is_dr) in enumerate(mms):
                                    if is_dr:
                                        nc.tensor.matmul(
                                            dst,
                                            w2f[:, :, gm * M + n * 128 : gm * M + n * 128 + 128],
                                            rh1f8[gm][:, :, :],
                                            start=(i == 0), stop=(i == len(mms) - 1),
                                            perf_mode=DR)
                                    else:
                                        nc.tensor.matmul(
                                            dst,
                                            w2b[:, gm * M + n * 128 : gm * M + n * 128 + 128],
                                            rh1f8[gm // 2][:, gm % 2, :],
                                            start=(i == 0), stop=(i == len(mms) - 1))
                        # op3: rh2 = relu(big)/(S1*S2) -> bf16 [128, 800]
                        rh2 = [pwork.tile([128, 800], bf16, tag=f"rh2{t}",
                                          name=f"rh2{t}") for t in range(2)]
                        for t in range(2):
                            nc.scalar.activation(
                                rh2[t][:, :].rearrange("p (m j) -> p m j", m=2),
                                big[t][:, :, 0:400],
                                AF.Relu, scale=1.0 / (S1 * S2))
                        # GEMM3: accumulate into pslogit rows 13b..13b+12
                        row = 13 * b + u
                        for kc in range(4):
                            nc.tensor.matmul(
                                pslogit[:, 0:400],
                                wp[:, kc * 105 + 52 - row : kc * 105 + 104 - row],
                                rh2[kc // 2][:, (kc % 2) * 400 : (kc % 2) * 400 + 400],
                                start=(nmm == 0), stop=(nmm == NMM_TOT - 1),
                                skip_group_check=True)
                            nmm += 1
                nc.scalar.activation(sig[:, :], pslogit[:, 0:400], AF.Sigmoid)
                nc.sync.dma_start(dout[:, :], sig[:, :])

    nc.compile()
    return nc


def _host_prep(x, pe_k, pe_v, comp, basis, root, rgcn_bias, W1, W2, Wp):
    import ml_dtypes

    bf = ml_dtypes.bfloat16
    f8 = ml_dtypes.float8_e4m3fn

    ra = _rel_adj(S) % NREL
    onehot = (ra[None, :, :] == np.arange(NREL)[:, None, None]).astype(np.float64)
    deg = onehot.sum(1)
    inv = np.where(deg > 0, 1.0 / np.maximum(deg, 1.0), 0.0)
    anorm = onehot * inv[:, None, :]
    ahat_full = np.einsum("rk,rij->kij", np.asarray(comp, np.float64), anorm)
    ahat_full = ahat_full.astype(np.float32)  # [2, S, S]
    pos = np.clip(np.arange(S)[:, None] - np.arange(S)[None, :] + 1, 0, MAXL)

    x = np.asarray(x, np.float32)
    W1 = np.asarray(W1, np.float32) * S1
    W1a, W1b = W1[:D], W1[D : D + P]
    W1c, W1d = W1[D + P : 2 * D + P], W1[2 * D + P :]

    W2s = np.asarray(W2, np.float32) * S2
    w2q = W2s.astype(f8)
    w2r = (W2s - w2q.astype(np.float32)).astype(f8)

    def pack_w2(w):  # [512, 512] -> [128, 2, 2*M]: [p, e, g*M + n]
        out = np.zeros((128, 2, 2 * M), w.dtype)
        for g in range(2):
            for e in range(2):
                out[:, e, g * M : (g + 1) * M] = w[g * 256 + e * 128 : g * 256 + e * 128 + 128]
        return np.ascontiguousarray(out)

    wp_np = np.asarray(Wp, np.float32)[:, 0]
    wpwin = np.zeros((128, 4 * 105), np.float32)
    for kc in range(4):
        wpwin[:, kc * 105 + 52] = wp_np[kc * 128 : kc * 128 + 128]

    # T table on host (S1 already folded into W1b/W1d)
    T = (np.asarray(pe_k, np.float32) @ W1b
         + np.asarray(pe_v, np.float32) @ W1d)  # [11, M]
    Tq = T.astype(bf).astype(np.float32)

    com = {
        "basis": np.concatenate(
            [_pack_k(np.asarray(basis[k], np.float32)) for k in range(2)],
            axis=1).astype(bf),
        "root": _pack_k(np.asarray(root, np.float32)).astype(bf),
        "w1a": _pack_k(W1a).astype(bf),
        "w1c": _pack_k(W1c).astype(bf),
        "w2q": pack_w2(w2q).reshape(128, 2 * 2 * M),
        "w2r": pack_w2(w2r).reshape(128, 2 * 2 * M),
        "w2b": np.ascontiguousarray(
            W2s.reshape(4, 128, M).transpose(1, 0, 2).reshape(128, 4 * M)).astype(bf),
        "wp": wpwin.astype(bf),
    }
    bias_p = np.zeros((128, 3), np.float32)
    rb = np.asarray(rgcn_bias, np.float32)
    for c in range(3):
        r = min(128, D - c * 128)
        bias_p[:r, c] = rb[c * 128 : c * 128 + r]
    com["bias"] = bias_p

    xt_all = x.transpose(2, 0, 1)  # [D, B, S]
    per_core = []
    for c in range(NCORES):
        i0 = c * IPC
        m = dict(com)
        xtc = np.empty((D, B * SC), np.float32)
        for b in range(B):
            xtc[:, b * SC : b * SC + S] = xt_all[:, b, :]
            xtc[:, b * SC + S : (b + 1) * SC] = xt_all[:, b, i0 : i0 + IPC]
        m["xT"] = xtc.astype(bf)
        ah = np.zeros((128, 4 * SC), np.float32)
        for k in range(2):
            for jc in range(2):
                r = 128 if jc == 0 else 72
                base = (k * 2 + jc) * SC
                ah[:r, base : base + S] = ahat_full[k, jc * 128 : jc * 128 + r, :]
                ah[:r, base + S : base + SC] = ahat_full[k, jc * 128 : jc * 128 + r, i0 : i0 + IPC]
        m["ahat"] = ah.astype(bf)
        # peR: [128, 2*FPAD] per g: [p, mslot*FPAD + pair] = Tq[pos, g*256+mslot*128+p]
        pr = Tq[pos[i0 : i0 + IPC, :].reshape(-1)]  # [5000, 512]
        prT = np.zeros((512, FPAD), np.float32)
        prT[:, :FPC] = pr.T
        for g in range(2):
            m[f"peR{g}"] = np.ascontiguousarray(
                prT.reshape(4, 128, FPAD)[2 * g : 2 * g + 2].transpose(1, 0, 2)
            ).astype(bf).reshape(128, 2 * FPAD)
        per_core.append(m)
    return per_core


def kernel(x, mask, pe_k, pe_v, comp, basis, root, rgcn_bias, W1, W2, Wp,
           _want_results=False, _trace=False):
    from concourse.bass_utils import run_bass_kernel_spmd

    if "nc" not in _prog_cache:
        _prog_cache["nc"] = _build_program()
    nc = _prog_cache["nc"]

    in_maps = _host_prep(x, pe_k, pe_v, comp, basis, root, rgcn_bias, W1, W2, Wp)
    res = run_bass_kernel_spmd(nc, in_maps, core_ids=list(range(NCORES)),
                               trace=_trace)

    out = np.zeros((B, S, S), np.float32)
    for c in range(NCORES):
        i0 = c * IPC
        rows = np.asarray(res.results[c]["out"], np.float32).reshape(4, 13, 400)
        for b in range(B):
            for u in range(NU - 1):
                out[b, i0 + 2 * u, :] = rows[b, u, :S]
                out[b, i0 + 2 * u + 1, :] = rows[b, u, S:]
            out[b, i0 + IPC - 1, :] = rows[b, NU - 1, :S]
    out *= np.asarray(mask, np.float32)
    if _want_results:
        return out, res
    return out


# revision 14
# speedup vs baseline: 1.1330x; 1.1330x over previous
"""Trainium2 Bass kernel for nn_CausePredictor (RGCN + pairwise MLP).

Sharding: data-parallel over the pairwise row index i (dim 1 of the
[B,S,S] output): 8 cores x 25 rows, replicated over B=4.  All per-core
differences are encoded as input DATA, so one SPMD program serves all
cores.

Math (matching reference.py):
  h   = sum_k Ahat_k.T @ (x[b] @ basis_k) + x[b] @ root + bias
  u   = h @ (W1a*S1)   v = h @ (W1c*S1)          # S1 = fp8 range scale
  T   = pe_k @ (W1b*S1) + pe_v @ (W1d*S1)        # [11, 512], host-built
  h1  = u[j] + v[i] + T[pos(i,j)]                # scaled by S1
  rh1 = relu(h1) -> fp8e4m3
  g2  = rh1 @ (W2*S2) via fp8 DoubleRow + hybrid fp8 residual weights
  rh2 = relu(g2)/(S1*S2) -> bf16
  out = sigmoid(rh2 @ Wp) * mask                 # mask applied on host

Stage B engine split: DVE does the u+peR adds (tensor_tensor, 800-wide
2x mode) and most relu(+v)->fp8 stores (tensor_scalar 2x), Pool (gpsimd)
takes the rest, Act drains GEMM2 PSUM (relu+rescale, 800-wide APs
spanning two banks of a [128,2,512] psum tile) and runs the per-batch
sigmoid.  GEMM3 accumulates all 52 (b,unit) rows into one [52,400]
PSUM bank via sliding-window column-placed Wp weights.
"""

import sys

sys.path.insert(0, "/opt/trn_rl_repo")

import numpy as np

B, S, D, M, P = 4, 200, 300, 512, 100
NREL, MAXL = 9, 10
NCORES = 8
IPC = S // NCORES  # 25 rows of i per core
NU = IPC // 2 + 1  # 13 units per b: 12x 2-row + 1x 1-row
FPC = IPC * S  # 5000 pairs per (b, core)
FPAD = FPC + 200  # peR padded so unit 12 reads in-bounds
SC = S + IPC  # 225

S1 = 16.0  # scale baked into W1 (u/v/T) so rh1 fills fp8 range
S2 = 32.0  # scale baked into W2

# Number of mc PAIRS (2x128 contraction each) run in bf16 instead of
# fp8-DoubleRow: 0 = all fp8 (fastest), 2 = all bf16 (most accurate).
FSPLIT = 0
# HILO: pair 0 feeds the two DoubleRow slots with fp8 hi+lo halves of the
# activation (12-bit effective), pair 1 stays plain fp8 + residual weights.
HILO = True

_prog_cache = {}

DCW = [128, 128, 44]  # D=300 chunks
JCW = [128, 72]  # S=200 chunks


def _rel_adj(s):
    ra = np.arange(s)[None, :] - np.arange(s)[:, None]
    for i in range(s):
        ra[i, i + 1 :] = 1
        num = 1
        for o in range(i - 1, -1, -2):
            ra[i, o] = -num
            if o - 1 >= 0:
                ra[i, o - 1] = -num
            num += 1
        ra[i, :i] = np.maximum(ra[i, :i], -8)
    return ra


def _pack_k(w):
    """[K, N] -> [128, ceil(K/128)*N], K chunked onto partitions."""
    k, n = w.shape
    nch = (k + 127) // 128
    out = np.zeros((128, nch * n), np.float32)
    for c in range(nch):
        r = min(128, k - c * 128)
        out[:r, c * n : c * n + n] = w[c * 128 : c * 128 + r]
    return out


def _build_program():
    import ml_dtypes  # noqa: F401
    import concourse.tile as tile
    from concourse import bacc, mybir

    f32 = mybir.dt.float32
    bf16 = mybir.dt.bfloat16
    f8 = mybir.dt.float8e4
    AF = mybir.ActivationFunctionType
    OP = mybir.AluOpType
    DR = mybir.MatmulPerfMode.DoubleRow

    nc = bacc.Bacc()

    dxT = nc.declare_dram_parameter("xT", [D, B * SC], bf16, isOutput=False)
    dahat = nc.declare_dram_parameter("ahat", [128, 4 * SC], bf16, isOutput=False)
    dbasis = nc.declare_dram_parameter("basis", [128, 1800], bf16, isOutput=False)
    droot = nc.declare_dram_parameter("root", [128, 900], bf16, isOutput=False)
    dbias = nc.declare_dram_parameter("bias", [128, 3], f32, isOutput=False)
    dw1a = nc.declare_dram_parameter("w1a", [128, 1536], bf16, isOutput=False)
    dw1c = nc.declare_dram_parameter("w1c", [128, 1536], bf16, isOutput=False)
    dw2q = nc.declare_dram_parameter("w2q", [128, 2 * 2 * M], f8, isOutput=False)
    dw2r = nc.declare_dram_parameter("w2r", [128, 2 * 2 * M], f8, isOutput=False)
    dw2b = nc.declare_dram_parameter("w2b", [128, 4 * M], bf16, isOutput=False)
    dw2d = nc.declare_dram_parameter("w2d", [128, 2 * 2 * M], f8, isOutput=False)
    dwp = nc.declare_dram_parameter("wp", [128, 4 * 105], bf16, isOutput=False)
    dpeR = [nc.declare_dram_parameter(f"peR{g}", [128, 2 * FPAD], bf16,
                                      isOutput=False) for g in range(2)]
    dout = nc.declare_dram_parameter("out", [52, 400], f32, isOutput=True)

    with tile.TileContext(nc) as tc:
        with (
            tc.tile_pool(name="persist", bufs=1) as pp,
            tc.tile_pool(name="work", bufs=3) as pwork,
        ):
            def load(name, shape, dt, src):
                t = pp.tile(shape, dt, tag=name, name=name)
                nc.sync.dma_start(t[tuple(slice(None) for _ in shape)], src)
                return t

            xT = [load(f"xT{c}", [DCW[c], B * SC], bf16,
                       dxT[c * 128 : c * 128 + DCW[c], :]) for c in range(3)]
            basis = load("basis", [128, 1800], bf16, dbasis[:, :])
            root = load("root", [128, 900], bf16, droot[:, :])
            bias = load("bias", [128, 3], f32, dbias[:, :])
            w1a = load("w1a", [128, 1536], bf16, dw1a[:, :])
            w1c = load("w1c", [128, 1536], bf16, dw1c[:, :])
            ahat = load("ahat", [128, 4 * SC], bf16, dahat[:, :])
            wp = load("wp", [128, 4 * 105], bf16, dwp[:, :])
            w2q = pp.tile([128, 2, 2 * M], f8, tag="w2q", name="w2q")
            nc.sync.dma_start(w2q[:, :, :], dw2q[:, :])
            w2r = pp.tile([128, 2, 2 * M], f8, tag="w2r", name="w2r")
            nc.sync.dma_start(w2r[:, :, :], dw2r[:, :])
            w2b = load("w2b", [128, 4 * M], bf16, dw2b[:, :])
            w2d = pp.tile([128, 2, 2 * M], f8, tag="w2d", name="w2d")
            nc.sync.dma_start(w2d[:, :, :], dw2d[:, :])
            peR = []
            for g in range(2):
                t = pp.tile([128, 2, FPAD], bf16, tag=f"peR{g}", name=f"peR{g}")
                nc.sync.dma_start(t[:, :, :], dpeR[g][:, :])
                peR.append(t)

            u2 = [[pp.tile([128, 2, S], bf16, tag=f"u2_{b}{g}", name=f"u2_{b}{g}")
                   for g in range(2)] for b in range(B)]
            vT = [[pp.tile([128, 26], f32, tag=f"vT{b}{mc}", name=f"vT{b}{mc}")
                   for mc in range(4)] for b in range(B)]
            hT = [[pp.tile([DCW[ec], SC], bf16, tag=f"hT{b}{ec}", name=f"hT{b}{ec}")
                   for ec in range(3)] for b in range(B)]
            t1 = [[[pp.tile([JCW[jc], D], bf16, tag=f"t1_{b}{k}{jc}",
                            name=f"t1_{b}{k}{jc}")
                    for jc in range(2)] for k in range(2)] for b in range(B)]
            sig = pp.tile([52, 400], f32, tag="sig", name="sig")

            # ---------------- stage A: t1 -> h -> u/v (all bf16) ----------
            with tc.tile_pool(name="psA", bufs=7, space="PSUM") as psA:
                for b in range(B):
                    for k in range(2):
                        for jc in range(2):
                            t1ps = psA.tile([JCW[jc], D], f32, tag="aps", name="t1ps")
                            for dc in range(3):
                                nc.tensor.matmul(
                                    t1ps[:, :],
                                    xT[dc][:, b * SC + jc * 128 : b * SC + jc * 128 + JCW[jc]],
                                    basis[0 : DCW[dc], (k * 3 + dc) * D : (k * 3 + dc) * D + D],
                                    start=(dc == 0), stop=(dc == 2),
                                )
                            if jc == 0:
                                nc.vector.tensor_copy(t1[b][k][jc][:, :], t1ps[:, :])
                            else:
                                nc.scalar.activation(t1[b][k][jc][:, :], t1ps[:, :],
                                                     AF.Copy)
                for b in range(B):
                    for ec in range(3):
                        hps = psA.tile([DCW[ec], SC], f32, tag="aps", name="hps")
                        first = True
                        for k in range(2):
                            for jc in range(2):
                                nc.tensor.matmul(
                                    hps[:, :],
                                    t1[b][k][jc][:, ec * 128 : ec * 128 + DCW[ec]],
                                    ahat[0 : JCW[jc], (k * 2 + jc) * SC : (k * 2 + jc + 1) * SC],
                                    start=first, stop=False)
                                first = False
                        for dc in range(3):
                            nc.tensor.matmul(
                                hps[:, :],
                                root[0 : DCW[dc], dc * D + ec * 128 : dc * D + ec * 128 + DCW[ec]],
                                xT[dc][:, b * SC : (b + 1) * SC],
                                start=False, stop=(dc == 2))
                        nc.scalar.activation(hT[b][ec][:, :], hps[:, :], AF.Identity,
                                             bias=bias[0 : DCW[ec], ec : ec + 1])
                for b in range(B):
                    for mc in range(4):
                        ups = psA.tile([128, SC], f32, tag="aps", name="ups")
                        for ec in range(3):
                            nc.tensor.matmul(
                                ups[:, 0:S],
                                w1a[0 : DCW[ec], ec * M + mc * 128 : ec * M + mc * 128 + 128],
                                hT[b][ec][:, 0:S], start=(ec == 0), stop=False)
                        for ec in range(3):
                            nc.tensor.matmul(
                                ups[:, S:SC],
                                w1c[0 : DCW[ec], ec * M + mc * 128 : ec * M + mc * 128 + 128],
                                hT[b][ec][:, S:SC], start=(ec == 0), stop=(ec == 2))
                        if mc % 2 == 0:
                            nc.vector.tensor_copy(u2[b][mc // 2][:, 0, :], ups[:, 0:S])
                        else:
                            nc.scalar.activation(u2[b][mc // 2][:, 1, :], ups[:, 0:S],
                                                 AF.Copy)
                        nc.vector.tensor_copy(vT[b][mc][:, 0:IPC], ups[:, S:SC])
                        nc.vector.memset(vT[b][mc][:, IPC:26], 0)

            # ---------------- stage B: the pairwise MLP ------------------
            with (
                tc.tile_pool(name="psB", bufs=3, space="PSUM") as psB,
                tc.tile_pool(name="psL", bufs=1, space="PSUM") as psL,
            ):
                pslogit = psL.tile([52, 512], f32, tag="pslogit", name="pslogit")
                nmm = 0
                prev = None
                NMM_TOT = B * NU * 4
                for b in range(B):
                    for u in range(NU):
                        nh = 2 if u < NU - 1 else 1
                        ncols = nh * 200
                        # op1: rh1a[g] = u2-broadcast + peR slice  [128, 800]
                        rh1a = [pwork.tile([128, 800], bf16, tag=f"rh1a{g}",
                                           name=f"rh1a{g}") for g in range(2)]
                        for g in range(2):
                            nc.vector.tensor_tensor(
                                rh1a[g][:, 0 : 2 * ncols].rearrange(
                                    "p (m r j) -> p m r j", m=2, r=nh),
                                u2[b][g][:, :, :].unsqueeze(2).broadcast_to(
                                    [128, 2, nh, S]),
                                peR[g][:, :, u * 400 : u * 400 + ncols].rearrange(
                                    "p m (r j) -> p m r j", r=nh),
                                OP.add)
                        # op2: rh1 = relu(rh1a + v), per (row, mc)
                        rh1f8 = [pwork.tile([128, 2, 400],
                                            bf16 if (HILO and g == 0) or g >= 2 - FSPLIT else f8,
                                            tag=f"rh1f8{g}", name=f"rh1f8{g}")
                                 for g in range(2)]
                        slot = 0
                        for g in range(2):
                            for m in range(2):
                                for h in range(nh):
                                    mc = g * 2 + m
                                    if HILO:
                                        eng = nc.vector if (g == 0 and m == 0) else nc.gpsimd
                                    else:
                                        eng = nc.vector if slot % 8 < 5 else nc.gpsimd
                                    eng.tensor_scalar(
                                        out=rh1f8[g][:, m, h * 200 : h * 200 + 200],
                                        in0=rh1a[g][:, m * ncols + h * 200 : m * ncols + h * 200 + 200],
                                        scalar1=vT[b][mc][:, 2 * u + h : 2 * u + h + 1],
                                        scalar2=0.0,
                                        op0=OP.add, op1=OP.max)
                                    slot += 1
                        if HILO:
                            # hi/lo fp8 split of pair 0: [hi0, lo0, hi1, lo1]
                            hilo = pwork.tile([128, 4, 400], f8, tag="hilo",
                                              name="hilo")
                            nc.vector.tensor_scalar(
                                out=hilo[:, 0:4:2, 0:ncols],
                                in0=rh1f8[0][:, :, 0:ncols],
                                scalar1=0.0, scalar2=None, op0=OP.add)
                            nc.vector.tensor_tensor(
                                hilo[:, 1:4:2, 0:ncols],
                                rh1f8[0][:, :, 0:ncols],
                                hilo[:, 0:4:2, 0:ncols],
                                OP.subtract)
                        # GEMM2 into [128, 2, 512] psum tiles (2 banks each)
                        big = [psB.tile([128, 2, 512], f32, tag="big", name="big")
                               for _ in range(2)]
                        for t in range(2):
                            for half in range(2):
                                n = t * 2 + half
                                dst = big[t][:, half, 0:ncols]
                                mms = []
                                if HILO:
                                    mms.append(("hilo", w2d, 0, hilo[:, 0:2, 0:ncols]))
                                    mms.append(("hilo", w2d, 1, hilo[:, 2:4, 0:ncols]))
                                    mms.append(("hilo", w2r, 0, hilo[:, 0:4:2, 0:ncols]))
                                    mms.append(("dr", w2q, 1, None))
                                    mms.append(("dr", w2r, 1, None))
                                else:
                                    for g in range(2):
                                        if g < 2 - FSPLIT:
                                            for w2f in (w2q, w2r):
                                                mms.append(("dr", w2f, g, None))
                                        else:
                                            for m in range(2):
                                                mms.append(("bf", None, g * 2 + m, None))
                                for i, (kind, w2f, gm, rhs) in enumerate(mms):
                                    if kind == "hilo":
                                        nc.tensor.matmul(
                                            dst,
                                            w2f[:, :, gm * M + n * 128 : gm * M + n * 128 + 128],
                                            rhs,
                                            start=(i == 0), stop=(i == len(mms) - 1),
                                            perf_mode=DR)
                                    elif kind == "dr":
                                        nc.tensor.matmul(
                                            dst,
                                            w2f[:, :, gm * M + n * 128 : gm * M + n * 128 + 128],
                                            rh1f8[gm][:, :, 0:ncols],
                                            start=(i == 0), stop=(i == len(mms) - 1),
                                            perf_mode=DR)
                                    else:
                                        nc.tensor.matmul(
                                            dst,
                                            w2b[:, gm * M + n * 128 : gm * M + n * 128 + 128],
                                            rh1f8[gm // 2][:, gm % 2, 0:ncols],
                                            start=(i == 0), stop=(i == len(mms) - 1))
                        # op3: rh2 = relu(big)/(S1*S2) -> bf16 [128, 800]
                        rh2 = [pwork.tile([128, 800], bf16, tag=f"rh2{t}",
                                          name=f"rh2{t}") for t in range(2)]
                        for t in range(2):
                            nc.scalar.activation(
                                rh2[t][:, 0 : 2 * ncols].rearrange(
                                    "p (m j) -> p m j", m=2),
                                big[t][:, :, 0:ncols],
                                AF.Relu, scale=1.0 / (S1 * S2))
                        # GEMM3 of the PREVIOUS unit (keeps PE queue moving)
                        if prev is not None:
                            p_row, p_ncols, p_rh2 = prev
                            for kc in range(4):
                                nc.tensor.matmul(
                                    pslogit[:, 0:p_ncols],
                                    wp[:, kc * 105 + 52 - p_row : kc * 105 + 104 - p_row],
                                    p_rh2[kc // 2][:, (kc % 2) * p_ncols : (kc % 2) * p_ncols + p_ncols],
                                    start=(nmm == 0), stop=(nmm == NMM_TOT - 1),
                                    skip_group_check=True)
                                nmm += 1
                        prev = (13 * b + u, ncols, rh2)
                p_row, p_ncols, p_rh2 = prev
                for kc in range(4):
                    nc.tensor.matmul(
                        pslogit[:, 0:p_ncols],
                        wp[:, kc * 105 + 52 - p_row : kc * 105 + 104 - p_row],
                        p_rh2[kc // 2][:, (kc % 2) * p_ncols : (kc % 2) * p_ncols + p_ncols],
                        start=(nmm == 0), stop=(nmm == NMM_TOT - 1),
                        skip_group_check=True)
                    nmm += 1
                nc.scalar.activation(sig[:, :], pslogit[:, 0:400], AF.Sigmoid)
                nc.sync.dma_start(dout[:, :], sig[:, :])

    nc.compile()
    return nc


def _host_prep(x, pe_k, pe_v, comp, basis, root, rgcn_bias, W1, W2, Wp):
    import ml_dtypes

    bf = ml_dtypes.bfloat16
    f8 = ml_dtypes.float8_e4m3fn

    ra = _rel_adj(S) % NREL
    onehot = (ra[None, :, :] == np.arange(NREL)[:, None, None]).astype(np.float64)
    deg = onehot.sum(1)
    inv = np.where(deg > 0, 1.0 / np.maximum(deg, 1.0), 0.0)
    anorm = onehot * inv[:, None, :]
    ahat_full = np.einsum("rk,rij->kij", np.asarray(comp, np.float64), anorm)
    ahat_full = ahat_full.astype(np.float32)  # [2, S, S]
    pos = np.clip(np.arange(S)[:, None] - np.arange(S)[None, :] + 1, 0, MAXL)

    x = np.asarray(x, np.float32)
    W1 = np.asarray(W1, np.float32) * S1
    W1a, W1b = W1[:D], W1[D : D + P]
    W1c, W1d = W1[D + P : 2 * D + P], W1[2 * D + P :]

    W2s = np.asarray(W2, np.float32) * S2
    w2q = W2s.astype(f8)
    w2r = (W2s - w2q.astype(np.float32)).astype(f8)

    def pack_w2(w):  # [512, 512] -> [128, 2, 2*M]: [p, e, g*M + n]
        out = np.zeros((128, 2, 2 * M), w.dtype)
        for g in range(2):
            for e in range(2):
                out[:, e, g * M : (g + 1) * M] = w[g * 256 + e * 128 : g * 256 + e * 128 + 128]
        return np.ascontiguousarray(out)

    wp_np = np.asarray(Wp, np.float32)[:, 0]
    wpwin = np.zeros((128, 4 * 105), np.float32)
    for kc in range(4):
        wpwin[:, kc * 105 + 52] = wp_np[kc * 128 : kc * 128 + 128]

    # T table on host (S1 already folded into W1b/W1d)
    T = (np.asarray(pe_k, np.float32) @ W1b
         + np.asarray(pe_v, np.float32) @ W1d)  # [11, M]
    Tq = T.astype(bf).astype(np.float32)

    com = {
        "basis": np.concatenate(
            [_pack_k(np.asarray(basis[k], np.float32)) for k in range(2)],
            axis=1).astype(bf),
        "root": _pack_k(np.asarray(root, np.float32)).astype(bf),
        "w1a": _pack_k(W1a).astype(bf),
        "w1c": _pack_k(W1c).astype(bf),
        "w2q": pack_w2(w2q).reshape(128, 2 * 2 * M),
        "w2d": np.ascontiguousarray(np.stack(
            [np.stack([w2q[mc * 128 : mc * 128 + 128]] * 2, axis=1)
             for mc in range(2)], axis=0).transpose(1, 2, 0, 3)
            .reshape(128, 2, 2 * M)).reshape(128, 2 * 2 * M),
        "w2r": pack_w2(w2r).reshape(128, 2 * 2 * M),
        "w2b": np.ascontiguousarray(
            W2s.reshape(4, 128, M).transpose(1, 0, 2).reshape(128, 4 * M)).astype(bf),
        "wp": wpwin.astype(bf),
    }
    bias_p = np.zeros((128, 3), np.float32)
    rb = np.asarray(rgcn_bias, np.float32)
    for c in range(3):
        r = min(128, D - c * 128)
        bias_p[:r, c] = rb[c * 128 : c * 128 + r]
    com["bias"] = bias_p

    xt_all = x.transpose(2, 0, 1)  # [D, B, S]
    per_core = []
    for c in range(NCORES):
        i0 = c * IPC
        m = dict(com)
        xtc = np.empty((D, B * SC), np.float32)
        for b in range(B):
            xtc[:, b * SC : b * SC + S] = xt_all[:, b, :]
            xtc[:, b * SC + S : (b + 1) * SC] = xt_all[:, b, i0 : i0 + IPC]
        m["xT"] = xtc.astype(bf)
        ah = np.zeros((128, 4 * SC), np.float32)
        for k in range(2):
            for jc in range(2):
                r = 128 if jc == 0 else 72
                base = (k * 2 + jc) * SC
                ah[:r, base : base + S] = ahat_full[k, jc * 128 : jc * 128 + r, :]
                ah[:r, base + S : base + SC] = ahat_full[k, jc * 128 : jc * 128 + r, i0 : i0 + IPC]
        m["ahat"] = ah.astype(bf)
        # peR: [128, 2*FPAD] per g: [p, mslot*FPAD + pair] = Tq[pos, g*256+mslot*128+p]
        pr = Tq[pos[i0 : i0 + IPC, :].reshape(-1)]  # [5000, 512]
        prT = np.zeros((512, FPAD), np.float32)
        prT[:, :FPC] = pr.T
        for g in range(2):
            m[f"peR{g}"] = np.ascontiguousarray(
                prT.reshape(4, 128, FPAD)[2 * g : 2 * g + 2].transpose(1, 0, 2)
            ).astype(bf).reshape(128, 2 * FPAD)
        per_core.append(m)
    return per_core


def kernel(x, mask, pe_k, pe_v, comp, basis, root, rgcn_bias, W1, W2, Wp,
           _want_results=False, _trace=False):
    from concourse.bass_utils import run_bass_kernel_spmd

    if "nc" not in _prog_cache:
        _prog_cache["nc"] = _build_program()
    nc = _prog_cache["nc"]

    in_maps = _host_prep(x, pe_k, pe_v, comp, basis, root, rgcn_bias, W1, W2, Wp)
    res = run_bass_kernel_spmd(nc, in_maps, core_ids=list(range(NCORES)),
                               trace=_trace)

    out = np.zeros((B, S, S), np.float32)
    for c in range(NCORES):
        i0 = c * IPC
        rows = np.asarray(res.results[c]["out"], np.float32).reshape(4, 13, 400)
        for b in range(B):
            for u in range(NU - 1):
                out[b, i0 + 2 * u, :] = rows[b, u, :S]
                out[b, i0 + 2 * u + 1, :] = rows[b, u, S:]
            out[b, i0 + IPC - 1, :] = rows[b, NU - 1, :S]
    out *= np.asarray(mask, np.float32)
    if _want_results:
        return out, res
    return out
